# revision 2
# baseline (speedup 1.0000x reference)
# Trainium2 Bass kernel for nn_ASTPruner (segment_reduce).
#
# Strategy: pure data parallel over batch (8 batches -> 8 NeuronCores).
# Device computes, per batch, the dense per-(t, token) reductions over C=768:
#   u = exp(x)            (ACT, fused accum -> Z = sum_c u)
#   S1 = sum_c u*x        (DVE scalar_tensor_tensor fused accumulate)
#   p = u / Z             (DVE tensor_scalar, written as float32r)
#   R[t] = M^T @ p        (PE matmul; M = region one-hot [196, 16])
#   w2 = p_t + p_{t+1};  S2 = sum_c w2*ln(w2)   (DVE add, ACT Ln, DVE STT)
#   w4 = w2_t + w2_{t+2}; S4 = sum_c w4*ln(w4)
# Host does the tiny tail (entropies from Z/S1/S2/S4, linear interp,
# min-max normalize, region entropy from R, kth-largest threshold, sigmoid).
import sys

sys.path.insert(0, '/opt/trn_rl_repo')
import numpy as np

B, T, N, C = 8, 32, 196, 768
RC, RF = 4, 8
NREG = 16  # 4 coarse + 8 fine, padded to 16
EPS = 1e-6
TAU = 1.0
TEMP = 0.1
ALPHA, BETA, GAMMA = 1.0, 0.5, 0.5
RHO = 0.5
LAM_TOKEN = LAM_HEAD = LAM_CH = LAM_BLOCK = 0.0
N_CORES = 8
HALVES = ((0, 128), (128, 68))  # (row offset, rows) per token half-tile

_NC = None


def _build_nc():
    import concourse.bacc as bacc
    import concourse.mybir as mybir
    import concourse.tile as tile

    f32 = mybir.dt.float32
    f32r = mybir.dt.float32r
    Alu = mybir.AluOpType
    Act = mybir.ActivationFunctionType

    nc = bacc.Bacc("TRN2", target_bir_lowering=False, debug=False,
                   enable_asserts=False, num_devices=N_CORES)
    x = nc.dram_tensor("x", [T * N, C], f32, kind="ExternalInput").ap()
    m = nc.dram_tensor("m", [N, NREG], f32, kind="ExternalInput").ap()
    zo = nc.dram_tensor("zo", [T, N], f32, kind="ExternalOutput").ap()
    s1o = nc.dram_tensor("s1o", [T, N], f32, kind="ExternalOutput").ap()
    s2o = nc.dram_tensor("s2o", [T - 1, N], f32, kind="ExternalOutput").ap()
    s4o = nc.dram_tensor("s4o", [T - 3, N], f32, kind="ExternalOutput").ap()
    ro = nc.dram_tensor("ro", [T, NREG, C], f32, kind="ExternalOutput").ap()

    with tile.TileContext(nc) as tc:
        with tc.tile_pool(name="const", bufs=1) as cpool, \
             tc.tile_pool(name="xin", bufs=3) as xpool, \
             tc.tile_pool(name="u", bufs=2) as upool, \
             tc.tile_pool(name="scr", bufs=2) as scrpool, \
             tc.tile_pool(name="p", bufs=6) as ppool, \
             tc.tile_pool(name="w2", bufs=10) as w2pool, \
             tc.tile_pool(name="w4", bufs=2) as w4pool, \
             tc.tile_pool(name="lw", bufs=2) as lwpool, \
             tc.tile_pool(name="small", bufs=4) as spool, \
             tc.tile_pool(name="rsb", bufs=2) as rpool, \
             tc.tile_pool(name="psA", bufs=2, space="PSUM") as psA, \
             tc.tile_pool(name="psB", bufs=2, space="PSUM") as psB:

            mt = []
            for h, (off, P) in enumerate(HALVES):
                t_ = cpool.tile([P, NREG], f32, tag=f"m{h}")
                nc.sync.dma_start(t_[:], m[off:off + P, :])
                mt.append(t_)

            p_ring = {}
            w2_ring = {}
            for t in range(T):
                ps0 = psA.tile([NREG, 512], f32)
                ps1 = psB.tile([NREG, 256], f32)
                p_ring[t] = []
                for h, (off, P) in enumerate(HALVES):
                    row = t * N + off
                    xt = xpool.tile([P, C], f32, tag=f"x{h}")
                    nc.sync.dma_start(xt[:], x[row:row + P, :])
                    u = upool.tile([P, C], f32, tag=f"u{h}")
                    z = spool.tile([P, 1], f32, tag=f"z{h}")
                    nc.scalar.activation(u[:], xt[:], Act.Exp, accum_out=z[:])
                    nc.sync.dma_start(zo[t, off:off + P], z[:])
                    rz = spool.tile([P, 1], f32, tag=f"rz{h}")
                    nc.vector.reciprocal(rz[:], z[:])
                    scr = scrpool.tile([P, C], f32, tag=f"scr{h}")
                    s1 = spool.tile([P, 1], f32, tag=f"s1{h}")
                    nc.vector.scalar_tensor_tensor(scr[:], u[:], 1.0, xt[:],
                                                   Alu.mult, Alu.mult,
                                                   accum_out=s1[:])
                    nc.sync.dma_start(s1o[t, off:off + P], s1[:])
                    p = ppool.tile([P, C], f32, tag=f"p{h}")
                    nc.vector.tensor_scalar_mul(p[:], u[:], rz[:])
                    p_ring[t].append(p)
                    nc.tensor.matmul(ps0[:], mt[h][:], p[:, 0:512],
                                     start=(h == 0), stop=(h == 1))
                    nc.tensor.matmul(ps1[:], mt[h][:], p[:, 512:768],
                                     start=(h == 0), stop=(h == 1))
                # PSUM -> SBUF -> HBM for region sums (copies on ScalarE)
                rs = rpool.tile([NREG, C], f32, tag="rs")
                nc.scalar.copy(rs[:, 0:512], ps0[:])
                nc.scalar.copy(rs[:, 512:768], ps1[:])
                nc.sync.dma_start(ro[t, :, :], rs[:])

                # window chain
                if t >= 1:
                    w2_ring[t - 1] = []
                    for h, (off, P) in enumerate(HALVES):
                        f32p = p_ring[t - 1][h][:]
                        f32q = p_ring[t][h][:]
                        w2 = w2pool.tile([P, C], f32, tag=f"w2{h}")
                        nc.vector.tensor_add(w2[:], f32p, f32q)
                        w2_ring[t - 1].append(w2)
                        lw2 = lwpool.tile([P, C], f32, tag=f"lw{h}")
                        nc.scalar.activation(lw2[:], w2[:], Act.Ln)
                        scr = scrpool.tile([P, C], f32, tag=f"scr{h}")
                        s2 = spool.tile([P, 1], f32, tag=f"s2{h}")
                        nc.vector.scalar_tensor_tensor(scr[:], w2[:], 1.0,
                                                       lw2[:], Alu.mult,
                                                       Alu.mult, accum_out=s2[:])
                        nc.sync.dma_start(s2o[t - 1, off:off + P], s2[:])
                    del p_ring[t - 1]
                if t >= 3:
                    for h, (off, P) in enumerate(HALVES):
                        w4 = w4pool.tile([P, C], f32, tag=f"w4{h}")
                        nc.vector.tensor_add(w4[:], w2_ring[t - 3][h][:],
                                             w2_ring[t - 1][h][:])
                        lw4 = lwpool.tile([P, C], f32, tag=f"lw{h}")
                        nc.scalar.activation(lw4[:], w4[:], Act.Ln)
                        scr = scrpool.tile([P, C], f32, tag=f"scr{h}")
                        s4 = spool.tile([P, 1], f32, tag=f"s4{h}")
                        nc.vector.scalar_tensor_tensor(scr[:], w4[:], 1.0,
                                                       lw4[:], Alu.mult,
                                                       Alu.mult, accum_out=s4[:])
                        nc.sync.dma_start(s4o[t - 3, off:off + P], s4[:])
                    del w2_ring[t - 3]
    nc.compile()
    return nc


def _get_nc():
    global _NC
    if _NC is None:
        _NC = _build_nc()
    return _NC


def _assign_regions(coords, centers):
    # float32 to match the reference bit-exactly
    c = coords.astype(np.float32)
    z = centers.astype(np.float32)
    diff = c[:, None, :] - z[None, :, :]
    d = np.sqrt((diff * diff).sum(-1))
    return np.argmin(d, axis=1)


def _interp_linear_T(H, T_out):
    T_in = H.shape[0]
    scale = T_in / T_out
    src = np.maximum((np.arange(T_out, dtype=np.float64) + 0.5) * scale - 0.5, 0.0)
    i0 = np.floor(src).astype(np.int64)
    i1 = np.minimum(i0 + 1, T_in - 1)
    w = src - i0
    return H[i0] * (1.0 - w)[:, None] + H[i1] * w[:, None]


def _normalize(H):
    mn, mx = H.min(), H.max()
    return (H - mn) / (mx - mn + EPS)


def _host_tail(Z, S1, S2, S4, R, rid_c, rid_f):
    H1 = np.log(Z) - S1 / Z - C * EPS
    H2 = np.log(2.0) - S2 / 2.0 - C * EPS
    H4 = np.log(4.0) - S4 / 4.0 - C * EPS
    H_time = (H1 + _interp_linear_T(H2, T) + _interp_linear_T(H4, T)) / 3.0

    cnt_c = np.bincount(rid_c, minlength=RC).astype(np.float64)
    cnt_f = np.bincount(rid_f, minlength=RF).astype(np.float64)
    pr_c = R[:, :RC, :] / (cnt_c + EPS)[None, :, None]
    pr_f = R[:, RC:RC + RF, :] / (cnt_f + EPS)[None, :, None]
    Hc = -(pr_c * np.log(pr_c + EPS)).sum(-1)
    Hf = -(pr_f * np.log(pr_f + EPS)).sum(-1)

    score = (ALPHA * _normalize(H_time) + BETA * _normalize(Hc)[:, rid_c]
             + GAMMA * _normalize(Hf)[:, rid_f])
    k = max(1, int(RHO * T * N))
    kth = np.sort(score.reshape(-1))[::-1][k - 1]
    return 1.0 / (1.0 + np.exp(-(score - kth) / TEMP))


def kernel(token_feat, centers_coarse, centers_fine, g_head, g_ch, g_block,
           patch_coords):
    from concourse import bass_utils

    token_feat = np.ascontiguousarray(np.asarray(token_feat, dtype=np.float32))
    rid_c = _assign_regions(np.asarray(patch_coords), np.asarray(centers_coarse))
    rid_f = _assign_regions(np.asarray(patch_coords), np.asarray(centers_fine))
    Mw = np.zeros((N, NREG), np.float32)
    Mw[np.arange(N), rid_c] = 1.0
    Mw[np.arange(N), RC + rid_f] = 1.0

    nc = _get_nc()
    in_maps = [{"x": token_feat[b].reshape(T * N, C), "m": Mw}
               for b in range(N_CORES)]
    res = bass_utils.run_bass_kernel_spmd(nc, in_maps,
                                          core_ids=list(range(N_CORES)))
    masks = []
    for b in range(B):
        r = res.results[b]
        masks.append(_host_tail(r["zo"].astype(np.float64),
                                r["s1o"].astype(np.float64),
                                r["s2o"].astype(np.float64),
                                r["s4o"].astype(np.float64),
                                r["ro"].astype(np.float64),
                                rid_c, rid_f))
    mask = np.stack(masks).astype(np.float32)

    head_w = (1.0 / (1.0 + np.exp(-np.asarray(g_head, np.float64)))).astype(np.float32)
    ch_w = (1.0 / (1.0 + np.exp(-np.asarray(g_ch, np.float64)))).astype(np.float32)
    block_w = (1.0 / (1.0 + np.exp(-np.asarray(g_block, np.float64)))).astype(np.float32)
    sparsity_token = 1.0 - mask.mean()
    L_AST = np.float32(LAM_TOKEN * sparsity_token
                       + LAM_HEAD * (1.0 - head_w.mean())
                       + LAM_CH * (1.0 - ch_w.mean())
                       + LAM_BLOCK * (1.0 - block_w.mean()))
    return mask, head_w, ch_w, block_w, L_AST


# revision 6
# speedup vs baseline: 1.0035x; 1.0035x over previous
# Trainium2 Bass kernel for nn_ASTPruner (segment_reduce).
#
# Strategy: pure data parallel over batch (8 batches -> 8 NeuronCores).
# Device computes, per batch, the dense per-(t, token) reductions over C=768:
#   u = exp(x)            (ACT, fused accum -> Z = sum_c u)
#   S1 = sum_c u*x        (DVE scalar_tensor_tensor fused accumulate)
#   p = u / Z             (DVE tensor_scalar, written as float32r)
#   R[t] = M^T @ p        (PE matmul; M = region one-hot [196, 16])
#   w2 = p_t + p_{t+1};  S2 = sum_c w2*ln(w2)   (DVE add, ACT Ln, DVE STT)
#   w4 = w2_t + w2_{t+2}; S4 = sum_c w4*ln(w4)
# Host does the tiny tail (entropies from Z/S1/S2/S4, linear interp,
# min-max normalize, region entropy from R, kth-largest threshold, sigmoid).
import sys

sys.path.insert(0, '/opt/trn_rl_repo')
import numpy as np

B, T, N, C = 8, 32, 196, 768
RC, RF = 4, 8
NREG = 16  # 4 coarse + 8 fine, padded to 16
EPS = 1e-6
TAU = 1.0
TEMP = 0.1
ALPHA, BETA, GAMMA = 1.0, 0.5, 0.5
RHO = 0.5
LAM_TOKEN = LAM_HEAD = LAM_CH = LAM_BLOCK = 0.0
N_CORES = 8
HALVES = ((0, 128), (128, 68))  # (row offset, rows) per token half-tile

_NC = None


def _build_nc():
    import concourse.bacc as bacc
    import concourse.mybir as mybir
    import concourse.tile as tile
    import concourse.hw_specs as hw_specs

    # Pin ACT to the one table set containing Exp, Ln and Copy so the
    # scheduler's exp/ln interleaving never triggers a table reload
    # (~2.7us per switch, observed 62 switches without this).
    _orig_tables = hw_specs.get_activation_tables

    def _only_lnexp(arch):
        t = _orig_tables(arch)
        # keep insertion order (act_func_set_id is the index); empty out all
        # other sets so the chooser can only pick natural_log_exp_and_others
        return {k: (v if k == "natural_log_exp_and_others" else set())
                for k, v in t.items()}

    bacc.get_activation_tables = _only_lnexp

    f32 = mybir.dt.float32
    f32r = mybir.dt.float32r
    Alu = mybir.AluOpType
    Act = mybir.ActivationFunctionType

    nc = bacc.Bacc("TRN2", target_bir_lowering=False, debug=False,
                   enable_asserts=False, num_devices=N_CORES)
    x = nc.dram_tensor("x", [T * N, C], f32, kind="ExternalInput").ap()
    m = nc.dram_tensor("m", [N, NREG], f32, kind="ExternalInput").ap()
    zo = nc.dram_tensor("zo", [T, N], f32, kind="ExternalOutput").ap()
    s1o = nc.dram_tensor("s1o", [T, N], f32, kind="ExternalOutput").ap()
    s2o = nc.dram_tensor("s2o", [T - 1, N], f32, kind="ExternalOutput").ap()
    s4o = nc.dram_tensor("s4o", [T - 3, N], f32, kind="ExternalOutput").ap()
    ro = nc.dram_tensor("ro", [T, NREG, C], f32, kind="ExternalOutput").ap()

    with tile.TileContext(nc) as tc:
        with tc.tile_pool(name="const", bufs=1) as cpool, \
             tc.tile_pool(name="xin", bufs=3) as xpool, \
             tc.tile_pool(name="u", bufs=2) as upool, \
             tc.tile_pool(name="scr", bufs=3) as scrpool, \
             tc.tile_pool(name="p", bufs=4) as ppool, \
             tc.tile_pool(name="w2", bufs=6) as w2pool, \
             tc.tile_pool(name="w4", bufs=2) as w4pool, \
             tc.tile_pool(name="lw", bufs=3) as lwpool, \
             tc.tile_pool(name="small", bufs=8) as spool, \
             tc.tile_pool(name="rsb", bufs=3) as rpool, \
             tc.tile_pool(name="psA", bufs=3, space="PSUM") as psA, \
             tc.tile_pool(name="psB", bufs=3, space="PSUM") as psB:

            mt = []
            for h, (off, P) in enumerate(HALVES):
                t_ = cpool.tile([P, NREG], f32, tag=f"m{h}")
                nc.sync.dma_start(t_[:], m[off:off + P, :])
                mt.append(t_)

            p_ring = {}
            w2_ring = {}
            for t in range(T):
                ps0 = psA.tile([NREG, 512], f32)
                ps1 = psB.tile([NREG, 256], f32)
                p_ring[t] = []
                for h, (off, P) in enumerate(HALVES):
                    row = t * N + off
                    xt = xpool.tile([P, C], f32, tag=f"x{h}")
                    nc.sync.dma_start(xt[:], x[row:row + P, :])
                    u = upool.tile([P, C], f32, tag=f"u{h}")
                    z = spool.tile([P, 1], f32, tag=f"z{h}")
                    nc.scalar.activation(u[:], xt[:], Act.Exp, accum_out=z[:])
                    nc.sync.dma_start(zo[t, off:off + P], z[:])
                    rz = spool.tile([P, 1], f32, tag=f"rz{h}")
                    nc.vector.reciprocal(rz[:], z[:])
                    scr = scrpool.tile([P, C], f32, tag=f"scr{h}")
                    s1 = spool.tile([P, 1], f32, tag=f"s1{h}")
                    nc.vector.scalar_tensor_tensor(scr[:], u[:], 1.0, xt[:],
                                                   Alu.mult, Alu.mult,
                                                   accum_out=s1[:])
                    nc.sync.dma_start(s1o[t, off:off + P], s1[:])
                    p = ppool.tile([P, C], f32, tag=f"p{h}")
                    nc.vector.tensor_scalar_mul(p[:], u[:], rz[:])
                    p_ring[t].append(p)
                    nc.tensor.matmul(ps0[:], mt[h][:], p[:, 0:512],
                                     start=(h == 0), stop=(h == 1))
                    nc.tensor.matmul(ps1[:], mt[h][:], p[:, 512:768],
                                     start=(h == 0), stop=(h == 1))
                # PSUM -> SBUF -> HBM for region sums (copies on ScalarE)
                rs = rpool.tile([NREG, C], f32, tag="rs")
                nc.scalar.copy(rs[:, 0:512], ps0[:])
                nc.scalar.copy(rs[:, 512:768], ps1[:])
                nc.sync.dma_start(ro[t, :, :], rs[:])

                # window chain
                if t >= 1:
                    w2_ring[t - 1] = []
                    for h, (off, P) in enumerate(HALVES):
                        f32p = p_ring[t - 1][h][:]
                        f32q = p_ring[t][h][:]
                        w2 = w2pool.tile([P, C], f32, tag=f"w2{h}")
                        nc.vector.tensor_add(w2[:], f32p, f32q)
                        w2_ring[t - 1].append(w2)
                        lw2 = lwpool.tile([P, C], f32, tag=f"lw{h}")
                        nc.scalar.activation(lw2[:], w2[:], Act.Ln)
                        scr = scrpool.tile([P, C], f32, tag=f"scr{h}")
                        s2 = spool.tile([P, 1], f32, tag=f"s2{h}")
                        nc.vector.scalar_tensor_tensor(scr[:], w2[:], 1.0,
                                                       lw2[:], Alu.mult,
                                                       Alu.mult, accum_out=s2[:])
                        nc.sync.dma_start(s2o[t - 1, off:off + P], s2[:])
                    del p_ring[t - 1]
                if t >= 3:
                    for h, (off, P) in enumerate(HALVES):
                        w4 = w4pool.tile([P, C], f32, tag=f"w4{h}")
                        nc.vector.tensor_add(w4[:], w2_ring[t - 3][h][:],
                                             w2_ring[t - 1][h][:])
                        lw4 = lwpool.tile([P, C], f32, tag=f"lw{h}")
                        nc.scalar.activation(lw4[:], w4[:], Act.Ln)
                        scr = scrpool.tile([P, C], f32, tag=f"scr{h}")
                        s4 = spool.tile([P, 1], f32, tag=f"s4{h}")
                        nc.vector.scalar_tensor_tensor(scr[:], w4[:], 1.0,
                                                       lw4[:], Alu.mult,
                                                       Alu.mult, accum_out=s4[:])
                        nc.sync.dma_start(s4o[t - 3, off:off + P], s4[:])
                    del w2_ring[t - 3]
    nc.compile()
    return nc


def _get_nc():
    global _NC
    if _NC is None:
        _NC = _build_nc()
    return _NC


def _assign_regions(coords, centers):
    # float32 to match the reference bit-exactly
    c = coords.astype(np.float32)
    z = centers.astype(np.float32)
    diff = c[:, None, :] - z[None, :, :]
    d = np.sqrt((diff * diff).sum(-1))
    return np.argmin(d, axis=1)


def _interp_linear_T(H, T_out):
    T_in = H.shape[0]
    scale = T_in / T_out
    src = np.maximum((np.arange(T_out, dtype=np.float64) + 0.5) * scale - 0.5, 0.0)
    i0 = np.floor(src).astype(np.int64)
    i1 = np.minimum(i0 + 1, T_in - 1)
    w = src - i0
    return H[i0] * (1.0 - w)[:, None] + H[i1] * w[:, None]


def _normalize(H):
    mn, mx = H.min(), H.max()
    return (H - mn) / (mx - mn + EPS)


def _host_tail(Z, S1, S2, S4, R, rid_c, rid_f):
    H1 = np.log(Z) - S1 / Z - C * EPS
    H2 = np.log(2.0) - S2 / 2.0 - C * EPS
    H4 = np.log(4.0) - S4 / 4.0 - C * EPS
    H_time = (H1 + _interp_linear_T(H2, T) + _interp_linear_T(H4, T)) / 3.0

    cnt_c = np.bincount(rid_c, minlength=RC).astype(np.float64)
    cnt_f = np.bincount(rid_f, minlength=RF).astype(np.float64)
    pr_c = R[:, :RC, :] / (cnt_c + EPS)[None, :, None]
    pr_f = R[:, RC:RC + RF, :] / (cnt_f + EPS)[None, :, None]
    Hc = -(pr_c * np.log(pr_c + EPS)).sum(-1)
    Hf = -(pr_f * np.log(pr_f + EPS)).sum(-1)

    score = (ALPHA * _normalize(H_time) + BETA * _normalize(Hc)[:, rid_c]
             + GAMMA * _normalize(Hf)[:, rid_f])
    k = max(1, int(RHO * T * N))
    kth = np.sort(score.reshape(-1))[::-1][k - 1]
    return 1.0 / (1.0 + np.exp(-(score - kth) / TEMP))


def kernel(token_feat, centers_coarse, centers_fine, g_head, g_ch, g_block,
           patch_coords):
    from concourse import bass_utils

    token_feat = np.ascontiguousarray(np.asarray(token_feat, dtype=np.float32))
    rid_c = _assign_regions(np.asarray(patch_coords), np.asarray(centers_coarse))
    rid_f = _assign_regions(np.asarray(patch_coords), np.asarray(centers_fine))
    Mw = np.zeros((N, NREG), np.float32)
    Mw[np.arange(N), rid_c] = 1.0
    Mw[np.arange(N), RC + rid_f] = 1.0

    nc = _get_nc()
    in_maps = [{"x": token_feat[b].reshape(T * N, C), "m": Mw}
               for b in range(N_CORES)]
    res = bass_utils.run_bass_kernel_spmd(nc, in_maps,
                                          core_ids=list(range(N_CORES)))
    masks = []
    for b in range(B):
        r = res.results[b]
        masks.append(_host_tail(r["zo"].astype(np.float64),
                                r["s1o"].astype(np.float64),
                                r["s2o"].astype(np.float64),
                                r["s4o"].astype(np.float64),
                                r["ro"].astype(np.float64),
                                rid_c, rid_f))
    mask = np.stack(masks).astype(np.float32)

    head_w = (1.0 / (1.0 + np.exp(-np.asarray(g_head, np.float64)))).astype(np.float32)
    ch_w = (1.0 / (1.0 + np.exp(-np.asarray(g_ch, np.float64)))).astype(np.float32)
    block_w = (1.0 / (1.0 + np.exp(-np.asarray(g_block, np.float64)))).astype(np.float32)
    sparsity_token = 1.0 - mask.mean()
    L_AST = np.float32(LAM_TOKEN * sparsity_token
                       + LAM_HEAD * (1.0 - head_w.mean())
                       + LAM_CH * (1.0 - ch_w.mean())
                       + LAM_BLOCK * (1.0 - block_w.mean()))
    return mask, head_w, ch_w, block_w, L_AST
